# revision 3
# baseline (speedup 1.0000x reference)
"""Bass/Trainium2 kernel for nn_Attention_73641509257513.

Reference op: y = LayerNorm(x; g_ln) -> per-head (H=8, head_dim=E=512)
causal attention -> output projection Wo. B=4, S=2048, E=512.

Sharding: Megatron tensor-parallel over heads - 1 head per NeuronCore
(8 cores). Each core:
  - LayerNorm of all tokens (replicated; g_ln folded into Wq/Wk/Wv on
    host; x shipped bf16; grouped DMAs, 1 in / 1 out per 4 token tiles)
  - xn transposed via DMA-xbar through a DRAM bounce; Q_T/K_T computed
    with fp8e4 DoubleRow matmuls (weights host-scaled x16 to dodge fp8
    subnormals, compensated in the exp scale), V in bf16
  - causal attention for its head, scores transposed S_T[k,q] via fp8
    DoubleRow matmuls; exp on ACT; causal mask = 128-col tensor multiply
    on the diagonal blocks only; softmax denominator via a ones[128,128]
    matmul whose output is broadcast across partitions, so 1/den applies
    per-column on ctx_T (no transpose of the denominator needed)
  - partial output Y_h = (ctx_T/den).T @ Wo_h
  - ReduceScatter(add) across the 8 cores in 4 pieces, each gated by
    late-batch rows (y_all interleaves batches) so the collectives fire
    only inside attention(2)/(3), after every DMA transpose is done:
    in-flight collectives serialize against DMA transposes and stall
    the queues otherwise. Final rs->y copies ride the gpsimd SWDGE.
Host reassembles the row-shards.

Scheduling: LN+transposes for batches 1,2 run inside attention(0), for
batch 3 inside attention(1). PSUM->SBUF copies alternate DVE/ACT. A
16-matmul warm-up on the weights lifts the PE clock-gate before the
first projection.
"""

import math

import numpy as np
import ml_dtypes

import concourse.bacc as bacc
import concourse.mybir as mybir
import concourse.tile as tile
from concourse.bass_utils import run_bass_kernel_spmd

B, S, E, H = 4, 2048, 512, 8
P = 128
EC = E // P            # 4 e-chunks
TT = S // P            # 16 token tiles per batch
QT = S // 512          # 4 q-tiles (512 wide) per batch
NCORES = 8
LN_EPS = 1e-5
SCALE = 1.0 / math.sqrt(E)
SHARD = S // NCORES    # 256 rows per batch per core

BF16 = mybir.dt.bfloat16
F32 = mybir.dt.float32
FP8 = mybir.dt.float8e4

OPTS = {
    "proj_copy": "dve",      # dve | act
    "ctx_copy": "act",       # dve | act
    "yscale": "act",         # dve | act
    "diag_first": True,
    "expt_bufs": 16,
    "rs_mode": "gated",  # gated | late | batch_last4 | one
    "ln_group": 4,
    "ps_sc_bufs": 3,
    "rs_dtype": "bf16",
    "ps_den_bufs": 1,
    "fp8_scores": True,
    "fp8_qkproj": True,
    "ln_dma": "act",         # act | sync: queue for x-in/xn-out DMAs
}

# Wq/Wk host-side scale (power of 2): keeps fp8 weights out of subnormals;
# compensated exactly in the exp() scale argument.
WSCALE = 16.0


def build_nc():
    nc = bacc.Bacc("TRN2", target_bir_lowering=False, debug=False,
                   num_devices=NCORES)
    x_in = nc.dram_tensor("x", [B * S, E], BF16, kind="ExternalInput").ap()
    wqk_dt_in = (mybir.dt.float8e4 if OPTS["fp8_qkproj"]
                 else mybir.dt.bfloat16)
    wq_in = nc.dram_tensor("wq", [E, E], wqk_dt_in, kind="ExternalInput").ap()
    wk_in = nc.dram_tensor("wk", [E, E], wqk_dt_in, kind="ExternalInput").ap()
    wv_in = nc.dram_tensor("wv", [E, E], BF16, kind="ExternalInput").ap()
    wo_in = nc.dram_tensor("wo", [E, E], BF16, kind="ExternalInput").ap()
    mask_in = nc.dram_tensor("trimask", [P, P], BF16,
                             kind="ExternalInput").ap()
    rsdt = BF16 if OPTS["rs_dtype"] == "bf16" else F32
    y_out = nc.dram_tensor("y", [B * SHARD, E], rsdt,
                           kind="ExternalOutput").ap()
    qk_dt = FP8 if OPTS["fp8_scores"] else BF16
    wqk_dt = FP8 if OPTS["fp8_qkproj"] else BF16
    # scores were computed from (WSCALE*Wq) and (WSCALE*Wk)
    exp_scale = SCALE / (WSCALE * WSCALE) if OPTS["fp8_qkproj"] else SCALE

    with tile.TileContext(nc) as tc:
        with (tc.tile_pool(name="consts", bufs=1) as consts,
              tc.tile_pool(name="ln", bufs=3) as ln,
              tc.tile_pool(name="lnst", bufs=4) as lnst,
              tc.tile_pool(name="xnt", bufs=3) as xnt,
              tc.tile_pool(name="xnt8", bufs=2) as xnt8p,
              tc.tile_pool(name="qkv", bufs=2) as qkv,
              tc.tile_pool(name="expt", bufs=OPTS["expt_bufs"]) as expt,
              tc.tile_pool(name="ctxp", bufs=2) as ctxp,
              tc.tile_pool(name="yout", bufs=6) as yout,
              tc.tile_pool(name="denp", bufs=2) as denp,
              tc.tile_pool(name="ps_mm", bufs=2, space="PSUM") as ps_mm,
              tc.tile_pool(name="ps_sc", bufs=OPTS["ps_sc_bufs"],
                           space="PSUM") as ps_sc,
              tc.tile_pool(name="ps_ctx", bufs=2, space="PSUM") as ps_ctx,
              tc.tile_pool(name="ps_den", bufs=OPTS["ps_den_bufs"],
                           space="PSUM") as ps_den,
              tc.tile_pool(name="dram", bufs=2, space="DRAM") as dram):

            # weights: [e_in, e_out] -> sbuf [128, EC, 512] (chunk c = rows 128c..)
            w_sb = {}
            for name, w_ap in (("wq", wq_in), ("wk", wk_in),
                               ("wv", wv_in), ("wo", wo_in)):
                dt = wqk_dt if name in ("wq", "wk") else BF16
                t = consts.tile([P, EC, E], dt, name=f"{name}_sb")
                nc.sync.dma_start(out=t, in_=w_ap.rearrange("(c p) n -> p c n", p=P))
                w_sb[name] = t
            ones_sb = consts.tile([P, P], BF16)
            nc.vector.memset(ones_sb, 1.0)
            mask_sb = consts.tile([P, P], BF16)
            nc.sync.dma_start(out=mask_sb, in_=mask_in)
            eps_sb = consts.tile([P, 1], F32)
            nc.vector.memset(eps_sb, LN_EPS)

            # PE warm-up: dummy matmuls on the weights as soon as they land,
            # so the HAM clock-gate is at full rate when real work arrives
            warm = ps_mm.tile([P, 512], F32, tag="mm", name="warm")
            for wi in range(16):
                nc.tensor.matmul(warm, w_sb["wv"][:, wi % EC, 0: P],
                                 w_sb["wv"][:, (wi + 1) % EC, :],
                                 start=(wi == 0), stop=(wi == 15))

            def alloc_xn_dram(b):
                return dram.tile([S, E], BF16, tag="xnd", name=f"xnd{b}")

            def emit_ln_load(b, t0, ng):
                """x-load DMA for token tiles t0..t0+ng of batch b."""
                row0 = b * S + P * t0
                g = OPTS["ln_group"]
                ln_eng = nc.scalar if OPTS["ln_dma"] == "act" else nc.sync
                x_grp = ln.tile([P, g, E], BF16, tag="x", name=f"x{b}_{t0}",
                                bufs=5)
                ln_eng.dma_start(
                    out=x_grp[:, :ng, :],
                    in_=x_in[row0: row0 + P * ng, :].rearrange(
                        "(c p) n -> p c n", p=P))
                return x_grp

            def emit_ln_norm(b, xn_dram, x_grp, t0, ng):
                """Stats + normalize + xn-store for a loaded group."""
                g = OPTS["ln_group"]
                ln_eng = nc.scalar if OPTS["ln_dma"] == "act" else nc.sync
                mv_all = lnst.tile([P, g, 2], F32, tag="mv",
                                   name=f"mv{b}_{t0}")
                for gi in range(ng):
                    stats = lnst.tile([P, nc.vector.BN_STATS_DIM], F32,
                                      tag="st", name=f"st{b}_{t0+gi}")
                    nc.vector.bn_stats(out=stats, in_=x_grp[:, gi, :])
                    nc.vector.bn_aggr(out=mv_all[:, gi, :], in_=stats)
                std_all = lnst.tile([P, g], F32, tag="sd", name=f"sd{b}_{t0}")
                nc.scalar.activation(std_all[:, :ng], mv_all[:, :ng, 1],
                                     mybir.ActivationFunctionType.Sqrt,
                                     bias=eps_sb)
                rs_all = lnst.tile([P, g], F32, tag="rs", name=f"rs{b}_{t0}")
                nc.vector.reciprocal(rs_all[:, :ng], std_all[:, :ng])
                xn_grp = ln.tile([P, g, E], BF16, tag="xn", name=f"xn{b}_{t0}")
                for gi in range(ng):
                    nc.vector.tensor_scalar(
                        xn_grp[:, gi, :], x_grp[:, gi, :], mv_all[:, gi, 0:1],
                        rs_all[:, gi: gi + 1],
                        mybir.AluOpType.subtract,
                        mybir.AluOpType.mult)
                ln_eng.dma_start(
                    out=xn_dram[P * t0: P * (t0 + ng), :].rearrange(
                        "(c p) n -> p c n", p=P),
                    in_=xn_grp[:, :ng, :])

            def alloc_xnt(b):
                xn_t = xnt.tile([P, EC, S], BF16, tag="xnt", name=f"xnt{b}")
                xn_t8 = None
                if OPTS["fp8_qkproj"]:
                    xn_t8 = xnt8p.tile([P, EC, S], FP8, tag="xnt8",
                                     name=f"xnt8_{b}")
                return xn_t, xn_t8

            def emit_transposes(b, xn_dram, xn_t, xn_t8, tg):
                """Transpose-load xn_dram rows [512tg, 512(tg+1)) into xn_t.

                DMA transposes serialize against in-flight collectives
                (framework constraint), so the schedule keeps all of these
                inside attention(0)/(1) and all collectives in (2)/(3)."""
                for c in range(EC):
                    nc.sync.dma_start_transpose(
                        out=xn_t[:, c, 512 * tg: 512 * (tg + 1)],
                        in_=xn_dram[512 * tg: 512 * (tg + 1),
                                    P * c: P * (c + 1)])

            def emit_proj(b, xn_t, xn_t8):
                """Q_T/K_T projections + V for batch b from xn_t."""
                qt_sb = qkv.tile([P, EC, S], qk_dt, tag="qt", name=f"qt{b}")
                kt_sb = qkv.tile([P, EC, S], qk_dt, tag="kt", name=f"kt{b}")
                v_sb = qkv.tile([P, TT, E], BF16, tag="v", name=f"v{b}")
                for tg in range(S // 512):
                    if xn_t8 is not None:
                        nc.vector.tensor_copy(
                            xn_t8[:, :, 512 * tg: 512 * (tg + 1)],
                            xn_t[:, :, 512 * tg: 512 * (tg + 1)])
                    for t in range(4 * tg, 4 * (tg + 1)):
                        ps = ps_mm.tile([P, 512], F32, tag="mm",
                                        name=f"pv{b}_{t}")
                        for ei in range(EC):
                            nc.tensor.matmul(ps,
                                             xn_t[:, ei, P * t: P * (t + 1)],
                                             w_sb["wv"][:, ei, :],
                                             start=(ei == 0), stop=(ei == EC - 1))
                        (nc.scalar.copy if t % 2 else
                         nc.vector.tensor_copy)(v_sb[:, t, :], ps)
                    for dst, wname in ((qt_sb, "wq"), (kt_sb, "wk")):
                        w = w_sb[wname]
                        for eo in range(EC):
                            ps = ps_mm.tile([P, 512], F32, tag="mm",
                                            name=f"pj{b}_{tg}_{wname}{eo}")
                            if OPTS["fp8_qkproj"]:
                                for pr in range(EC // 2):
                                    nc.tensor.matmul(
                                        ps,
                                        w[:, 2 * pr: 2 * pr + 2,
                                          P * eo: P * (eo + 1)],
                                        xn_t8[:, 2 * pr: 2 * pr + 2,
                                              512 * tg: 512 * (tg + 1)],
                                        start=(pr == 0),
                                        stop=(pr == EC // 2 - 1),
                                        perf_mode=mybir.MatmulPerfMode.DoubleRow)
                            else:
                                for ei in range(EC):
                                    nc.tensor.matmul(
                                        ps,
                                        w[:, ei, P * eo: P * (eo + 1)],
                                        xn_t[:, ei, 512 * tg: 512 * (tg + 1)],
                                        start=(ei == 0), stop=(ei == EC - 1))
                            (nc.scalar.copy if eo % 2 else
                             nc.vector.tensor_copy)(
                                dst[:, eo, 512 * tg: 512 * (tg + 1)], ps)
                return qt_sb, kt_sb, v_sb

            def emit_attn(b, qt_sb, kt_sb, v_sb, y_all, ln_hook=None,
                          rs_hook=None):
                """Causal attention + output projection into y_all."""
                for qt in range(QT):
                    if ln_hook is not None:
                        ln_hook(qt)
                    q0 = 512 * qt
                    nkc = (q0 + 512) // P
                    # diagonal (masked) chunks first so their mask-multiplies
                    # complete while the dense chunks' scores still stream
                    if OPTS["diag_first"]:
                        kcs = list(range(4 * qt, nkc)) + list(range(4 * qt))
                    else:
                        kcs = list(range(nkc))
                    ex_by_kc = {}
                    off_by_kc = {}
                    for kc in kcs:
                        # diagonal chunk j covers keys q0+128j..: queries below
                        # 128j are fully masked, so shrink N to the valid range
                        j = kc - 4 * qt
                        off = P * j if j > 0 else 0
                        w = 512 - off
                        off_by_kc[kc] = off
                        ps_s = ps_sc.tile([P, 512], F32, tag="sc",
                                          name=f"sc{b}_{qt}_{kc}")
                        if OPTS["fp8_scores"]:
                            for pr in range(EC // 2):
                                nc.tensor.matmul(
                                    ps_s[:, off:],
                                    kt_sb[:, 2 * pr: 2 * pr + 2,
                                          P * kc: P * (kc + 1)],
                                    qt_sb[:, 2 * pr: 2 * pr + 2,
                                          q0 + off: q0 + 512],
                                    start=(pr == 0), stop=(pr == EC // 2 - 1),
                                    perf_mode=mybir.MatmulPerfMode.DoubleRow)
                        else:
                            for ei in range(EC):
                                nc.tensor.matmul(
                                    ps_s[:, off:],
                                    kt_sb[:, ei, P * kc: P * (kc + 1)],
                                    qt_sb[:, ei, q0 + off: q0 + 512],
                                    start=(ei == 0), stop=(ei == EC - 1))
                        ex = expt.tile([P, 512], BF16, tag="ex",
                                       name=f"ex{b}_{qt}_{kc}")
                        nc.scalar.activation(ex[:, off:], ps_s[:, off:],
                                             mybir.ActivationFunctionType.Exp,
                                             scale=exp_scale)
                        if j >= 0:
                            # only the first 128 query-cols of the chunk hit
                            # the triangle; the rest are fully unmasked
                            nc.vector.tensor_mul(ex[:, off: off + P],
                                                 ex[:, off: off + P],
                                                 mask_sb)
                        ex_by_kc[kc] = ex

                    # ones[128,128] matmul: den broadcast to every
                    # partition, so 1/den applies per-column on ctx_T and
                    # the y-scale (and its DMA transpose) disappears
                    ps_d = ps_den.tile([P, 512], F32, tag="den",
                                       name=f"dn{b}_{qt}")
                    for i, kc in enumerate(kcs):
                        o = off_by_kc[kc]
                        nc.tensor.matmul(ps_d[:, o:], ones_sb,
                                         ex_by_kc[kc][:, o:],
                                         start=(i == 0), stop=(i == nkc - 1))
                    recip = denp.tile([P, 512], F32, tag="rcp",
                                      name=f"rc{b}_{qt}")
                    nc.vector.reciprocal(recip, ps_d)

                    ctx_sb = ctxp.tile([P, EC, 512], BF16, tag="ctx",
                                       name=f"cx{b}_{qt}")
                    for ec in range(EC):
                        ps_c = ps_ctx.tile([P, 512], F32, tag="ctx",
                                           name=f"pc{b}_{qt}_{ec}")
                        for i, kc in enumerate(kcs):
                            o = off_by_kc[kc]
                            nc.tensor.matmul(
                                ps_c[:, o:],
                                v_sb[:, kc, P * ec: P * (ec + 1)],
                                ex_by_kc[kc][:, o:],
                                start=(i == 0), stop=(i == nkc - 1))
                        nc.vector.tensor_mul(ctx_sb[:, ec, :], ps_c, recip)

                    for jj in range(4):
                        ps_y = ps_mm.tile([P, 512], F32, tag="mm",
                                          name=f"py{b}_{qt}_{jj}")
                        for ec in range(EC):
                            nc.tensor.matmul(
                                ps_y,
                                ctx_sb[:, ec, P * jj: P * (jj + 1)],
                                w_sb["wo"][:, ec, :],
                                start=(ec == 0), stop=(ec == EC - 1))
                        y_tile = yout.tile([P, 512], rsdt, tag="yt",
                                           name=f"yt{b}_{qt}_{jj}")
                        nc.scalar.copy(y_tile, ps_y)
                        if OPTS["rs_mode"] == "gated":
                            if b < 3:
                                r0 = qt * 1536 + b * 512 + jj * P
                            else:
                                r0 = 6144 + qt * 512 + jj * P
                        else:
                            r0 = b * S + q0 + P * jj
                        nc.sync.dma_start(out=y_all[r0: r0 + P, :], in_=y_tile)
                    if rs_hook is not None:
                        rs_hook(qt)

            rs_state = {"out_r0": 0}

            def emit_rs(y_all, r0, nrows):
                """ReduceScatter y_all[r0:r0+nrows]; final copy deferred."""
                out_r0 = rs_state["out_r0"]
                rs_state["out_r0"] += nrows // NCORES
                rs_o = dram.tile([nrows // NCORES, E], rsdt, tag="rso",
                                 bufs=8, name=f"rs{r0}")
                nc.gpsimd.collective_compute(
                    "ReduceScatter", mybir.AluOpType.add,
                    replica_groups=[list(range(NCORES))],
                    ins=[y_all[r0: r0 + nrows, :].rearrange("s e -> (s e)")],
                    outs=[rs_o.rearrange("s e -> (s e)")])
                # deferred: the y_out copy would park in the sync queue
                # waiting on the collective, blocking everything behind it
                rs_state.setdefault("copies", []).append(
                    (rs_o, out_r0, nrows // NCORES))

            def emit_rs_copies():
                # on the gpsimd SWDGE: these wait on their collective, and
                # would head-of-line block a hardware DGE queue
                for rs_o, out_r0, n in rs_state.get("copies", []):
                    nc.gpsimd.dma_start(out=y_out[out_r0: out_r0 + n, :],
                                        in_=rs_o)

            # pipeline: LN+transposes for batches 1,2 run inside attention(0)
            # and for batch 3 inside attention(1); every collective runs
            # inside attention(2)/(3) where no DMA transposes remain (they
            # would serialize against each other).
            y_all = dram.tile([B * S, E], rsdt, tag="yall", bufs=1)
            xn_d = {0: alloc_xn_dram(0)}
            xnt_all = {0: alloc_xnt(0)}
            b0_grps = [emit_ln_load(0, 4 * tg, 4) for tg in range(4)]
            for tg in range(4):
                emit_ln_norm(0, xn_d[0], b0_grps[tg], 4 * tg, 4)
                emit_transposes(0, xn_d[0], *xnt_all[0], tg)

            def make_ln_hook(bs):
                def hook(qt):
                    grps = []
                    for nb in bs:
                        if nb not in xn_d:
                            xn_d[nb] = alloc_xn_dram(nb)
                            xnt_all[nb] = alloc_xnt(nb)
                        grps.append(emit_ln_load(nb, 4 * qt, 4))
                    for nb, g in zip(bs, grps):
                        emit_ln_norm(nb, xn_d[nb], g, 4 * qt, 4)
                        emit_transposes(nb, xn_d[nb], *xnt_all[nb], qt)
                return hook

            if OPTS["rs_mode"] == "gated":
                # LN/transposes all done by end of attention(1); collectives
                # fire from attention(2) on, gated by batch-2/3 y rows.
                # 4 big pieces: per-collective fixed cost is ~15us
                ln_hooks = {0: make_ln_hook([1, 2]), 1: make_ln_hook([3])}
                rs_plan = {(2, 1): [(0, 3072)], (2, 3): [(3072, 3072)],
                           (3, 1): [(6144, 1024)], (3, 3): [(7168, 1024)]}
            elif OPTS["rs_mode"] == "late":
                ln_hooks = {b: make_ln_hook([b + 1]) for b in range(B - 1)}
                rs_plan = {(3, 0): [(0, S), (3 * S, 512)],
                           (3, 1): [(S, S), (3 * S + 512, 512)],
                           (3, 2): [(2 * S, S), (3 * S + 1024, 512)],
                           (3, 3): [(3 * S + 1536, 512)]}
            elif OPTS["rs_mode"] == "batch_last4":
                ln_hooks = {b: make_ln_hook([b + 1]) for b in range(B - 1)}
                rs_plan = {(b, 3): [(b * S, S)] for b in range(B - 1)}
                for qt in range(4):
                    rs_plan.setdefault((3, qt), []).append(
                        (3 * S + 512 * qt, 512))
            else:  # "one"
                ln_hooks = {b: make_ln_hook([b + 1]) for b in range(B - 1)}
                rs_plan = {(3, 3): [(0, B * S)]}

            for b in range(B):
                pk = emit_proj(b, *xnt_all[b])

                def rs_hook(qt, _b=b):
                    for (r0, n) in rs_plan.get((_b, qt), []):
                        emit_rs(y_all, r0, n)
                emit_attn(b, *pk, y_all, ln_hook=ln_hooks.get(b),
                          rs_hook=rs_hook)
            emit_rs_copies()

    nc.finalize()
    return nc


_NC_CACHE = None


def _get_nc():
    global _NC_CACHE
    if _NC_CACHE is None:
        _NC_CACHE = build_nc()
    return _NC_CACHE


def make_in_maps(inputs):
    """Host-side sharding: slice/cast per-core weights, fold g_ln."""
    x = np.asarray(inputs["x"], dtype=np.float32)
    g_ln = np.asarray(inputs["g_ln"], dtype=np.float32)
    Wq = np.asarray(inputs["Wq"], dtype=np.float32)
    Wk = np.asarray(inputs["Wk"], dtype=np.float32)
    Wv = np.asarray(inputs["Wv"], dtype=np.float32)
    Wo = np.asarray(inputs["Wo"], dtype=np.float32)

    x2 = np.ascontiguousarray(x.reshape(B * S, E)).astype(ml_dtypes.bfloat16)
    g = g_ln[:, None]
    tri = (np.arange(P)[None, :] >= np.arange(P)[:, None]).astype(
        ml_dtypes.bfloat16)
    wqk_np = ml_dtypes.float8_e4m3 if OPTS["fp8_qkproj"] else ml_dtypes.bfloat16
    ws = WSCALE if OPTS["fp8_qkproj"] else 1.0
    in_maps = []
    for h in range(NCORES):
        sl = slice(E * h, E * (h + 1))
        in_maps.append({
            "x": x2,
            "wq": np.ascontiguousarray(ws * g * Wq[:, sl]).astype(wqk_np),
            "wk": np.ascontiguousarray(ws * g * Wk[:, sl]).astype(wqk_np),
            "wv": np.ascontiguousarray(g * Wv[:, sl]).astype(ml_dtypes.bfloat16),
            "wo": np.ascontiguousarray(Wo[sl, :]).astype(ml_dtypes.bfloat16),
            "trimask": tri,
        })
    return in_maps


def _pieces():
    if OPTS["rs_mode"] == "gated":
        return [(0, 3072), (3072, 3072), (6144, 1024), (7168, 1024)]
    if OPTS["rs_mode"] == "late":
        return [(0, S), (3 * S, 512), (S, S), (3 * S + 512, 512),
                (2 * S, S), (3 * S + 1024, 512), (3 * S + 1536, 512)]
    if OPTS["rs_mode"] == "batch_last4":
        return [(0, S), (S, S), (2 * S, S)] + \
            [(3 * S + 512 * i, 512) for i in range(4)]
    elif OPTS["rs_mode"] == "one":
        return [(0, B * S)]
    raise ValueError(OPTS["rs_mode"])


def kernel(**inputs) -> np.ndarray:
    in_maps = make_in_maps(inputs)
    nc = _get_nc()
    res = run_bass_kernel_spmd(nc, in_maps, core_ids=list(range(NCORES)))

    y2 = np.empty((B * S, E), dtype=np.float32)
    if OPTS["rs_mode"] == "gated":
        # y_all layout: rows r<6144: (qt=r//1536, b=(r%1536)//512,
        # rr=r%512) -> b*S + qt*512 + rr; rows r>=6144: batch 3,
        # qt=(r-6144)//512 -> 3*S + (r-6144)
        out_r0 = 0
        for (pr0, pn) in _pieces():
            sh = pn // NCORES
            data = np.concatenate(
                [res.results[c]["y"][out_r0: out_r0 + sh, :] for c in
                 range(NCORES)]).astype(np.float32)
            r = np.arange(pr0, pr0 + pn)
            rows = np.where(
                r < 6144,
                ((r % 1536) // 512) * S + (r // 1536) * 512 + (r % 512),
                3 * S + (r - 6144))
            y2[rows] = data
            out_r0 += sh
        return y2.reshape(B, S, E)
    out_r0 = 0
    for (pr0, pn) in _pieces():
        sh = pn // NCORES
        for c in range(NCORES):
            y2[pr0 + sh * c: pr0 + sh * (c + 1), :] = \
                res.results[c]["y"][out_r0: out_r0 + sh, :].astype(np.float32)
        out_r0 += sh
    return y2.reshape(B, S, E)
